# revision 4
# baseline (speedup 1.0000x reference)
"""Trainium2 Bass kernel: batched single-head attention.

Reference computation (per batch b):
    q = x @ Wq + bq ; k = x @ Wk + bk ; v = x @ Wv + bv      # [S, H]
    out = softmax((q k^T) / sqrt(H)) @ v                     # [S, H]

Shapes: B=4, S=4096, D_IN=512, D_H=64, fp32.

Sharding: 8 cores = (batch, query-half). Core c handles batch c//2,
queries (c%2)*2048 .. +2048 (host rotates x[b] so its queries are rows
0:2048; softmax over keys is permutation-invariant). Host pre-transposes
to x^T [512, 4096].

The AV matmul runs as fp8e4 DoubleRow — P^T is written
by the ACT directly as fp8, V natural tiles are stored fp8 padded to 96
rows (DoubleRow lhsT free dim must be a multiple of 32; row 64 = ones
for the softmax denominator, rows 65:96 inert), and one DoubleRow
matmul consumes a key-tile PAIR (256 keys) per 512-query chunk. Job
grid = (kt-pair t, query chunk qc); four [96,512] psum accumulators
(one bank each), the qc3 bank borrowed by the projection/transpose
extras until job 24.

On-device dataflow per core:
  KV^T[128,s]   = [Wk|Wv]^T x^T + [bk;bv]     (PE->psum, DVE bias add)
  Q^T [64,2048] = Wq^T x^T + bq
  vnat8[128,kt,96] fp8 = PE-transpose of V^T rows; col 64 = ones
  per job (t, qc):  pst[128,1024] = S^T for kts (2t,2t+1) x 512 queries
                    pt8[128,1024] fp8 = exp(0.125 * pst)       (ACT)
                    out_qc[96,512] += DoubleRow(vnat8 pair t, pt8 pair)
  out rows 0:64 = numerator^T, row 64 = denominator; host divides.
"""

import numpy as np

B, S, D_IN, D_H = 4, 4096, 512, 64
QW = S // 2          # queries per core
N_CORES = 8
NKT = S // 128       # 32 key tiles
NTP = NKT // 2       # 16 key-tile pairs
NQC = QW // 512      # 4 query chunks of 512
NSC = S // 512       # 8 s chunks of 512
NDT = D_IN // 128    # 4 contraction tiles


def make_jobs():
    """(t, qc) order, chunk-arrival paced. The head only does s-chunk 0
    work, so early jobs use pairs t0/t1 x qc0/qc1 while later chunks
    stream in; qc2/qc3 defer until their q-projections (extras) land.
    ALL qc3 jobs sit past the last extra (job 24) because the extras
    borrow qc3's psum out bank."""
    jobs = [(0, 0), (1, 0), (0, 1), (1, 1),
            (2, 0), (2, 1), (3, 0), (3, 1),
            (0, 2), (1, 2), (2, 2), (3, 2)]
    for t in range(4, 7):
        jobs += [(t, 0), (t, 1), (t, 2)]
    jobs += [(7, 0), (7, 1), (7, 2), (7, 3)]
    for t in range(8, 16):
        jobs += [(t, 0), (t, 1), (t, 2), (t, 3)]
    jobs += [(t, 3) for t in range(7)]
    assert len(jobs) == NTP * NQC
    assert min(i for i, j in enumerate(jobs) if j[1] == 3) == 24
    return jobs


def build_nc(repeats=1, pad_iters=0, loop_iters=0):
    """Build + compile the Bacc module for one core (SPMD across 8)."""
    import concourse.bass as bass
    import concourse.tile as tile
    from concourse import bacc, mybir

    f32 = mybir.dt.float32
    f32r = mybir.dt.float32r
    f8 = mybir.dt.float8e4
    EXP = mybir.ActivationFunctionType.Exp
    DR = mybir.MatmulPerfMode.DoubleRow

    nc = bacc.Bacc("TRN2", target_bir_lowering=False, debug=False,
                   num_devices=N_CORES)

    xT_d = nc.dram_tensor("xT", (D_IN, S), f32r, kind="ExternalInput").ap()
    w_d = nc.dram_tensor("w", (D_IN, 192), f32r, kind="ExternalInput").ap()
    cst_d = nc.dram_tensor("consts", (128, 132), f32r,
                           kind="ExternalInput").ap()
    yT_d = nc.dram_tensor("yT", (65, QW), f32, kind="ExternalOutput").ap()

    jobs = make_jobs()
    # last job index per qc (for the AV stop flag + output ship)
    last_job_of_qc = {qc: max(i for i, j in enumerate(jobs) if j[1] == qc)
                      for qc in range(NQC)}
    first_job_of_qc = {qc: min(i for i, j in enumerate(jobs) if j[1] == qc)
                       for qc in range(NQC)}

    with tile.TileContext(nc) as tc:
        import contextlib
        with contextlib.ExitStack() as ctx:
            sb = ctx.enter_context(tc.tile_pool(name="sb", bufs=1))
            ptp = ctx.enter_context(tc.tile_pool(name="ptp", bufs=4))

            # ---- persistent buffers ----
            w_sb = sb.tile([128, NDT, 192], f32r)      # [Wk|Wv|Wq] d-tiles
            cst_sb = sb.tile([128, 132], f32r)         # eye|ones|pad|bkv|bq
            xt = sb.tile([128, NDT, S], f32r)          # x^T tiles
            kvt = sb.tile([128, S], f32r)              # 0:64 K^T, 64:128 V^T
            qt_sb = sb.tile([128, QW], f32r)           # rows 0:64 Q^T
            vnat8 = sb.tile([128, NKT, 96], f8)        # V fp8 | ones | pad
            yT_sb = sb.tile([128, QW], f32)
            warm_sb = sb.tile([128, 4], f32)
            pad_sb = (sb.tile([128, 8192], f32, name="pad_sb")
                      if pad_iters else None)

            id_sb = cst_sb[:, 0:128]
            bkv_sb = cst_sb[:, 130:131].bitcast(f32)
            bq_sb = cst_sb[:, 131:132].bitcast(f32)

            for _rep in range(repeats):
              with tc.tile_pool(name=f"pa{_rep}", bufs=1, space="PSUM") as pa:
                _bctx = contextlib.ExitStack()
                if loop_iters:
                    _bctx.enter_context(tc.For_i(0, loop_iters))
                # DMA queue order = completion order: weights, consts,
                # then x^T chunks 0..7.
                nc.sync.dma_start(w_sb, w_d.rearrange("(t p) m -> p t m",
                                                      p=128))
                nc.sync.dma_start(cst_sb, cst_d)
                xT_r = xT_d.rearrange("(t p) s -> p t s", p=128)
                for c in range(NSC):
                    cs = slice(512 * c, 512 * (c + 1))
                    nc.sync.dma_start(xt[:, :, cs], xT_r[:, :, cs])

                # warm-ups: pre-touch operands one semaphore at a time
                # (walrus allows at most ONE sync wait per engine instr)
                nc.scalar.activation(warm_sb[0:1, 2:3],
                                     cst_sb[0:1, 129:130].bitcast(f32),
                                     EXP, scale=1.0)
                nc.vector.tensor_copy(warm_sb[:, 0:1], bkv_sb)
                # vnat8 ones column (fp8(1.0) exact): broadcast resident col
                ones_col = bass.AP(tensor=cst_sb.tensor, offset=cst_sb.offset
                                   + 128, ap=[[132, 128], [0, NKT], [0, 32]])
                nc.vector.tensor_copy(vnat8[:, :, 64:96], ones_col)
                warm = pa.tile([128, 132], f32, tag="st", bufs=2)
                nc.tensor.matmul(warm[:, 0:2], lhsT=w_sb[:, 0, 0:128],
                                 rhs=w_sb[:, 0, 0:2], start=True, stop=True)
                nc.tensor.transpose(warm[0:1, 4:132].bitcast(f32r),
                                    in_=id_sb[:, 0:1], identity=id_sb)
                # HAM warm-up: junk matmuls keep PE busy through the x^T DMA
                # wait so the first S^T matmuls run at 2.4 GHz
                for _ in range(12):
                    nc.tensor.matmul(warm[:, 0:128], lhsT=w_sb[:, 0, 0:128],
                                     rhs=w_sb[:, 0, 0:128], start=True,
                                     stop=True)

                def proj_kv(c, tag="st"):
                    cs = slice(512 * c, 512 * (c + 1))
                    pkv = pa.tile([128, 1024], f32, tag=tag,
                                  bufs=(2 if tag == "st" else 1), name="pkv")
                    for dt in range(NDT):
                        nc.tensor.matmul(
                            pkv[:, 0:512],
                            lhsT=w_sb[:, dt, 0:128], rhs=xt[:, dt, cs],
                            start=(dt == 0), stop=(dt == NDT - 1))
                    nc.vector.tensor_scalar_add(kvt[:, cs], pkv[:, 0:512],
                                                bkv_sb)

                def proj_q(c, tag="st"):
                    cs = slice(512 * c, 512 * (c + 1))
                    pq = pa.tile([128, 512], f32, tag=tag,
                                 bufs=(2 if tag == "st" else 1), name="pq")
                    for dt in range(NDT):
                        nc.tensor.matmul(
                            pq[0:D_H, 0:512],
                            lhsT=w_sb[:, dt, 128:192], rhs=xt[:, dt, cs],
                            start=(dt == 0), stop=(dt == NDT - 1))
                    nc.vector.tensor_scalar_add(
                        qt_sb[0:D_H, cs], pq[0:D_H, 0:512], bq_sb[0:D_H, :])

                def v_nat(c, tag="st"):
                    pvt = pa.tile([128, 1024], f32r, tag=tag,
                                  bufs=(2 if tag == "st" else 1), name="pvt")
                    for j in range(4):
                        kt = 4 * c + j
                        nc.tensor.transpose(
                            pvt[:, D_H * j:D_H * (j + 1)],
                            in_=kvt[64:128, 128 * kt:128 * (kt + 1)],
                            identity=id_sb[64:128, 64:128])
                    nc.vector.tensor_copy(
                        vnat8[:, 4 * c:4 * (c + 1), 0:D_H],
                        pvt[:, 0:4 * D_H].rearrange("p (t h) -> p t h", h=D_H))
                    # junk matmul: advances the PE engine clock past the vnat
                    # copy's DVE tick (walrus 1-wait limit on later AV MMs)
                    nc.tensor.matmul(
                        pvt[0:96, 0:2].bitcast(f32),
                        lhsT=vnat8[:, 4 * c, :],
                        rhs=vnat8[:, 4 * c, 0:2],
                        start=True, stop=True)

                def chunk_work(c):
                    # kv projection + V transpose of one s-chunk, in the
                    # qc3 out bank (all qc3 AV jobs come after the extras)
                    cs = slice(512 * c, 512 * (c + 1))
                    t = pa.tile([128, 512], f32, tag="outq3", bufs=1,
                                name="cw")
                    for dt in range(NDT):
                        nc.tensor.matmul(
                            t[:, 0:512],
                            lhsT=w_sb[:, dt, 0:128], rhs=xt[:, dt, cs],
                            start=(dt == 0), stop=(dt == NDT - 1))
                    nc.vector.tensor_scalar_add(kvt[:, cs], t[:, 0:512],
                                                bkv_sb)
                    for j in range(4):
                        kt = 4 * c + j
                        nc.tensor.transpose(
                            t[:, D_H * j:D_H * (j + 1)].bitcast(f32r),
                            in_=kvt[64:128, 128 * kt:128 * (kt + 1)],
                            identity=id_sb[64:128, 64:128])
                    nc.vector.tensor_copy(
                        vnat8[:, 4 * c:4 * (c + 1), 0:D_H],
                        t[:, 0:4 * D_H].bitcast(f32r)
                        .rearrange("p (t h) -> p t h", h=D_H))
                    nc.tensor.matmul(
                        t[0:96, 300:302],
                        lhsT=vnat8[:, 4 * c, :],
                        rhs=vnat8[:, 4 * c, 0:2],
                        start=True, stop=True)

                # head: s-chunk 0 only (jobs 0-3 use pairs t0/t1 x qc0/1);
                # everything else streams in as extras.
                proj_kv(0)
                v_nat(0)
                proj_q(0)

                # out accumulators: one [96, 512] bank per query chunk
                # (rows 0:64 AV^T, row 64 denominator, 65:96 junk pad).
                # qc3's bank is borrowed by the extras until job 24.
                pouts = {qc: pa.tile([96, 512], f32, tag=f"outq{qc}",
                                     name=f"pout{qc}")
                         for qc in range(3)}

                def st_tile(t, qc):
                    # S^T psum tile for kt pair (2t, 2t+1) x query chunk qc
                    pst = pa.tile([128, 1024], f32, tag="st", bufs=2,
                                  name=f"pst_{t}_{qc}")
                    qs = slice(512 * qc, 512 * (qc + 1))
                    for i in range(2):
                        kt = 2 * t + i
                        nc.tensor.matmul(
                            pst[:, 512 * i:512 * (i + 1)],
                            lhsT=kvt[0:64, 128 * kt:128 * (kt + 1)],
                            rhs=qt_sb[0:64, qs],
                            start=True, stop=True)
                    return pst

                # extra work interleaved into early jobs, deadline-paced:
                # chunk c (keys) is needed by the first job with t//2 == c,
                # i.e. job index >= 8c roughly; q-chunks 2,3 by their first
                # qc use.
                extra_at = {
                    0: lambda: proj_q(1, tag="outq3"),
                    1: lambda: chunk_work(1),
                    3: lambda: proj_q(2, tag="outq3"),
                    5: lambda: chunk_work(2),
                    7: lambda: proj_q(3, tag="outq3"),
                    9: lambda: chunk_work(3),
                    13: lambda: chunk_work(4),
                    17: lambda: chunk_work(5),
                    20: lambda: chunk_work(6),
                    24: lambda: chunk_work(7),
                }

                njobs = len(jobs)
                psts = {0: st_tile(*jobs[0]), 1: st_tile(*jobs[1])}
                for j in range(njobs):
                    t, qc = jobs[j]
                    pt8 = ptp.tile([128, 1024], f8, tag="pt", name="pt8")
                    nc.scalar.activation(pt8, psts.pop(j), EXP, scale=0.125)
                    if j in extra_at:
                        extra_at.pop(j)()
                    if j + 2 < njobs:
                        psts[j + 2] = st_tile(*jobs[j + 2])
                    # AV: one DoubleRow fp8 matmul per job
                    if qc not in pouts:
                        pouts[qc] = pa.tile([96, 512], f32, tag=f"outq{qc}",
                                            name=f"pout{qc}")
                    rhs_pair = bass.AP(
                        tensor=pt8.tensor, offset=pt8.offset,
                        ap=[[pt8.ap[0][0], 128], [512, 2], [1, 512]])
                    nc.tensor.matmul(
                        pouts[qc],
                        lhsT=vnat8[:, 2 * t:2 * (t + 1), :],
                        rhs=rhs_pair,
                        start=(j == first_job_of_qc[qc]),
                        stop=(j == last_job_of_qc[qc]),
                        perf_mode=DR, skip_group_check=True)
                    if j == last_job_of_qc[qc]:
                        qs = slice(512 * qc, 512 * (qc + 1))
                        nc.vector.tensor_copy(yT_sb[0:65, qs],
                                              pouts[qc][0:65, :])
                        nc.sync.dma_start(yT_d[:, qs], yT_sb[0:65, qs])
                assert not extra_at
                _bctx.close()

            if pad_iters:
                # timing pad: fixed-length busy loop so the NEFF body
                # exceeds the relay's latency-hiding window; identical
                # between the K=1 and K=N timing NEFFs so it cancels.
                with tc.For_i(0, pad_iters):
                    for _ in range(8):
                        nc.scalar.activation(pad_sb, pad_sb, EXP, scale=0.0)

    nc.compile()
    return nc


def _prep_core_inputs(c, x, Wq, bq, Wk, bk, Wv, bv):
    b, qh = c // 2, c % 2
    xb = x[b]
    if qh:
        xb = np.concatenate([xb[QW:], xb[:QW]], axis=0)
    consts = np.zeros((128, 132), np.float32)
    consts[:, 0:128] = np.eye(128, dtype=np.float32)
    consts[:, 128] = 1.0                      # vnat ones column
    consts[:, 130] = np.concatenate([bk, bv])  # [bk;bv] per-partition bias
    consts[0:D_H, 131] = bq
    return {
        "xT": np.ascontiguousarray(xb.T),
        "w": np.ascontiguousarray(np.concatenate([Wk, Wv, Wq], axis=1)),
        "consts": consts,
    }


def gather_output(per_core_yT):
    """per_core_yT: list of 8 arrays [65, QW] -> full y [B, S, D_H]."""
    y = np.empty((B, S, D_H), np.float32)
    for c in range(N_CORES):
        b, qh = c // 2, c % 2
        yT = np.asarray(per_core_yT[c])
        y[b, qh * QW:(qh + 1) * QW] = (yT[0:D_H] / yT[D_H:D_H + 1]).T
    return y


def run(x, Wq, bq, Wk, bk, Wv, bv, trace=False):
    """Returns (y [B,S,H], BassKernelResults)."""
    from concourse import bass_utils

    x = np.asarray(x, np.float32)
    in_maps = [
        _prep_core_inputs(c, x, np.asarray(Wq, np.float32),
                          np.asarray(bq, np.float32),
                          np.asarray(Wk, np.float32),
                          np.asarray(bk, np.float32),
                          np.asarray(Wv, np.float32),
                          np.asarray(bv, np.float32))
        for c in range(N_CORES)
    ]
    nc = build_nc()
    res = bass_utils.run_bass_kernel_spmd(
        nc, in_maps, core_ids=list(range(N_CORES)), trace=trace)
    y = gather_output([res.results[c]["yT"] for c in range(N_CORES)])
    return y, res


def kernel(x, Wq, bq, Wk, bk, Wv, bv):
    y, _ = run(x, Wq, bq, Wk, bk, Wv, bv, trace=False)
    return y
